# revision 18
# baseline (speedup 1.0000x reference)
"""LIF spiking-neuron kernel for Trainium2 (Bass/Tile), 8-core SPMD.

Problem: x [B=32, T=8, C=128, H=32, W=32] f32.  Per (b,c,h,w) neuron,
sequential over T:
    mem = mem*TAU + x_t;  spike = (mem - 1 > 0);  mem = 0 if spike
TAU = 0.5, THRESH = 1.0.

Sharding: batch dim B=32 split 4-per-core across 8 NeuronCores; the
recurrence is along T only, so there is no communication.

Bit-exact algorithm vs the fp32 reference:
  TAU = 0.5 is a power of two, so rescale the state M_t = 2^t * m_t.
  The decay becomes a pure add:  M_t = M_{t-1} + y_t with y_t = 2^t*x_t
  (prescaled exactly on the HOST - power-of-2 scaling commutes with fp
  rounding, so every M_t is bit-exactly 2^t * m_t).
  spike_t = (M_t > 2^t): computed as Sign(M*2^-t - 1) -> u8 on ACT
  (power-of-2 scale exact; saturating cast maps Sign's -1 to 0;
  verified on HW).

The recurrence is elementwise, so it column-splits across engines with
no interaction.  Fat [C=128, B_loc*H*W=4096] tiles per step; DVE owns
columns [0:SG), gpsimd owns [SG:4096) (both verified bit-exact on HW):
  DVE:  acc  M' = y_t + M (tensor_tensor add)
        rst  M'' = (M' <= 2^t)*M' (fused stt, maskless)
  GPS:  acc  TT add; rst = is_le mask (u8) + TT mult (2 ops)
        + u8 output DMA descriptors (SWDGE queue, emitted one step
        late so they never wait on a fresh compare)
  ACT:  spike compare, off the recurrence path
  DMA:  per-t input split in halves across the SP/ACT HWDGE queues in
        deadline order; ~410 GB/s aggregate observed.

HBM traffic: 16 MiB in + 4 MiB u8 out per core (cast to f32 on host).
"""

import numpy as np

from concourse import bacc, bass, mybir, tile
from concourse.alu_op_type import AluOpType
from concourse.bass_utils import run_bass_kernel_spmd

# Full-problem shape (hardcoded per harness contract).
B, T, C, H, W = 32, 8, 128, 32, 32
N_CORES = 8
B_LOC = B // N_CORES          # 4 batches per core
F = H * W                     # 1024 free elements per batch
FW = B_LOC * F                # 4096 free elements per fat tile
SG = 3 * F                    # DVE columns [0:SG), gpsimd [SG:FW)
FP32 = mybir.dt.float32
U8 = mybir.dt.uint8

_NC_CACHE = {}


def _emit(tc, y_d, o_d):
    nc = tc.nc
    SIGN = mybir.ActivationFunctionType.Sign

    with (
        tc.tile_pool(name="xp", bufs=8) as xp,
        tc.tile_pool(name="sp", bufs=3) as sp,
        tc.tile_pool(name="np_", bufs=2) as np_,
        tc.tile_pool(name="mp", bufs=1) as mp,
    ):
        ms = [mp.tile([C, FW], FP32, name=f"m{i}") for i in range(3)]
        m_prev = None
        pend_out = None  # delayed output DMA: (s_tile, t, halves)
        dv, gp = slice(0, SG), slice(SG, FW)

        def flush_out():
            if pend_out is None:
                return
            s_, t_, nb0, nb = pend_out
            nc.gpsimd.dma_start(
                out=o_d[nb0 : nb0 + nb, t_].rearrange("b c h w -> c b (h w)"),
                in_=s_.rearrange("c (b f) -> c b f", b=nb),
            )

        for t in range(T):
            th = float(2.0**t)
            xt = xp.tile([C, FW], FP32)
            m_cur = ms[t % 3]
            if t == 0:
                # per-batch chunk DMAs; compute starts on the first 512 KiB
                for b in range(B_LOC):
                    eng = nc.sync if b % 2 == 0 else nc.scalar
                    eng.dma_start(
                        out=xt[:, b * F : (b + 1) * F],
                        in_=y_d[b, 0].rearrange("c h w -> c (h w)"),
                    )
                    bs = slice(b * F, (b + 1) * F)
                    if b < 3:  # DVE columns: fused copy+reset
                        nc.vector.scalar_tensor_tensor(
                            m_cur[:, bs], xt[:, bs], 1.0, xt[:, bs],
                            AluOpType.is_le, AluOpType.mult,
                        )
                    else:      # gpsimd columns: is_le mask + mult
                        n = np_.tile([C, F], U8)
                        nc.gpsimd.tensor_single_scalar(
                            n, xt[:, bs], 1.0, AluOpType.is_le
                        )
                        nc.gpsimd.tensor_tensor(
                            m_cur[:, bs], xt[:, bs], n, AluOpType.mult
                        )
                pre = xt
            else:
                # input halves across both HWDGE queues, deadline order
                for h in range(2):
                    eng = nc.sync if h == 0 else nc.scalar
                    eng.dma_start(
                        out=xt[:, h * FW // 2 : (h + 1) * FW // 2].rearrange(
                            "c (b f) -> c b f", b=2
                        ),
                        in_=y_d[2 * h : 2 * h + 2, t].rearrange(
                            "b c h w -> c b (h w)"
                        ),
                    )
                # accumulate M' = y_t + M, column-split DVE / gpsimd
                nc.vector.tensor_tensor(
                    m_cur[:, dv], xt[:, dv], m_prev[:, dv], AluOpType.add
                )
                nc.gpsimd.tensor_tensor(
                    m_cur[:, gp], xt[:, gp], m_prev[:, gp], AluOpType.add
                )
                pre = m_cur
            # spike u8, single ACT pass (off the recurrence path)
            halves = 2 if t == T - 1 else 1
            HS = FW // halves
            ss = []
            for h in range(halves):
                s = sp.tile([C, HS], U8, name=f"s{halves}")
                nc.scalar.activation(
                    s, pre[:, h * HS : (h + 1) * HS], SIGN,
                    bias=-1.0, scale=1.0 / th,
                )
                ss.append(s)
            if t < T - 1:
                # reset into the next ping-pong tile, column-split
                m_rst = ms[(t + 1) % 3]
                nc.vector.scalar_tensor_tensor(
                    m_rst[:, dv], m_cur[:, dv], th, m_cur[:, dv],
                    AluOpType.is_le, AluOpType.mult,
                )
                n = np_.tile([C, FW - SG], U8)
                nc.gpsimd.tensor_single_scalar(
                    n, m_cur[:, gp], th, AluOpType.is_le
                )
                nc.gpsimd.tensor_tensor(
                    m_rst[:, gp], m_cur[:, gp], n, AluOpType.mult
                )
                m_prev = m_rst
            # emit the PREVIOUS step's output descriptors now (the compare
            # they wait on is long done - no gpsimd head-of-line stall)
            flush_out()
            if halves == 1:
                pend_out = (ss[0], t, 0, B_LOC)
            else:  # t == T-1: flush both halves immediately
                for h in range(halves):
                    pend_out = (ss[h], t, h * 2, 2)
                    flush_out()
                pend_out = None
        flush_out()


def build_nc():
    """Build + compile the per-core Bass program (cached)."""
    if "nc" in _NC_CACHE:
        return _NC_CACHE["nc"]
    nc = bacc.Bacc(
        "TRN2",
        target_bir_lowering=False,
        debug=False,
        enable_asserts=False,
        num_devices=N_CORES,
    )
    y_d = nc.dram_tensor("y", [B_LOC, T, C, H, W], FP32, kind="ExternalInput").ap()
    o_d = nc.dram_tensor("out", [B_LOC, T, C, H, W], U8, kind="ExternalOutput").ap()
    # register the -1.0 bias constant (memset in the preamble)
    th_t = nc.alloc_sbuf_tensor("const-float32--1.0", [C, 1], FP32)
    nc.gpsimd.memset(th_t.ap(), -1.0)
    nc.const_aps.aps[(FP32, -1.0)] = th_t.ap()
    with tile.TileContext(nc) as tc:
        _emit(tc, y_d, o_d)
    nc.compile()
    _NC_CACHE["nc"] = nc
    return nc


_POW2 = (2.0 ** np.arange(T, dtype=np.float32))[None, :, None, None, None]


def make_in_maps(x: np.ndarray) -> list[dict[str, np.ndarray]]:
    assert x.shape == (B, T, C, H, W) and x.dtype == np.float32, (x.shape, x.dtype)
    y = x * _POW2  # exact power-of-2 prescale on the host
    return [
        {"y": np.ascontiguousarray(y[i * B_LOC : (i + 1) * B_LOC])}
        for i in range(N_CORES)
    ]


def kernel(x: np.ndarray) -> np.ndarray:
    x = np.asarray(x, dtype=np.float32)
    nc = build_nc()
    res = run_bass_kernel_spmd(nc, make_in_maps(x), list(range(N_CORES)))
    return np.concatenate([r["out"] for r in res.results], axis=0).astype(np.float32)


# revision 21
# speedup vs baseline: 2.8024x; 2.8024x over previous
"""LIF spiking-neuron kernel for Trainium2 (Bass/Tile), 8-core SPMD.

Problem: x [B=32, T=8, C=128, H=32, W=32] f32.  Per (b,c,h,w) neuron,
sequential over T:
    mem = mem*TAU + x_t;  spike = (mem - 1 > 0);  mem = 0 if spike
TAU = 0.5, THRESH = 1.0.

Sharding: batch dim B=32 split 4-per-core across 8 NeuronCores; the
recurrence is along T only, so there is no communication.

Per-core algorithm (bit-exact vs the fp32 reference):
  TAU = 0.5 is a power of two, so rescale the state M_t = 2^t * m_t.
  The decay becomes a pure add:  M_t = M_{t-1} + 2^t * x_t  (the 2^t
  prescale of x is exact in fp32, and power-of-2 scaling commutes with
  fp rounding, so every M_t is bit-exactly 2^t * m_t).
  spike_t = (M_t > 2^t)  <=>  (m_t > 1)  <=>  reference's (m_t - 1 > 0).

The recurrence is DVE-serial (acc + reset per step, ~62 us of DVE at
123 G elem/s fp32); everything else hides behind it:
  DVE:  t=0   M = (x_0 <= 1)*x_0          (fused stt is_le+mult)
        t>=1  M' = x_t*2^t + M            (fused prescale+acc stt)
        t<=6  M'' = (M' <= 2^t)*M'        (fused maskless reset stt)
  ACT:  s_t u8 = Sign(M'*2^-t - 1): power-of-2 scale is exact, the
        saturating fp32->u8 cast maps Sign's -1 to 0 (verified on HW);
        single pass, off the recurrence path.
  DMA:  deadline-ordered input schedule so DVE never waits:
        t0+t1 in per-batch quarters interleaved on the two HWDGE
        queues (compute starts on the first 512 KiB), t2 whole on the
        gpsimd SWDGE queue, t3..t7 in halves across both HWDGE queues.
        u8 outputs on SWDGE behind t2 (never block input prefetch).

HBM traffic: 16 MiB in + 4 MiB u8 out per core (cast to f32 on host).
"""

import numpy as np

from concourse import bacc, bass, mybir, tile
from concourse.alu_op_type import AluOpType
from concourse.bass_utils import run_bass_kernel_spmd

# Full-problem shape (hardcoded per harness contract).
B, T, C, H, W = 32, 8, 128, 32, 32
N_CORES = 8
B_LOC = B // N_CORES          # 4 batches per core
F = H * W                     # 1024 free elements per batch
FW = B_LOC * F                # 4096 free elements per fat tile
FP32 = mybir.dt.float32
U8 = mybir.dt.uint8

_NC_CACHE = {}


def _emit(tc, x_d, o_d):
    nc = tc.nc
    SIGN = mybir.ActivationFunctionType.Sign

    def bslice(xt, b):
        return xt[:, b * F : (b + 1) * F]

    with (
        tc.tile_pool(name="xp", bufs=8) as xp,
        tc.tile_pool(name="sp", bufs=3) as sp,
        tc.tile_pool(name="mp", bufs=1) as mp,
    ):
        ms = [mp.tile([C, FW], FP32, name=f"m{i}") for i in range(3)]
        xts = [xp.tile([C, FW], FP32, name="xt") for _ in range(T)]

        # ---- input DMA triggers, deadline order ----------------------
        # t0+t1 quarters interleaved (sync: b0,b2; scalar: b1,b3)
        for t in (0, 1):
            for b in range(B_LOC):
                eng = nc.sync if b % 2 == 0 else nc.scalar
                eng.dma_start(
                    out=bslice(xts[t], b),
                    in_=x_d[b, t].rearrange("c h w -> c (h w)"),
                )
        # t2 whole on the gpsimd SWDGE queue (lands ~16us, needed ~20us)
        nc.gpsimd.dma_start(
            out=xts[2].rearrange("c (b f) -> c b f", b=B_LOC),
            in_=x_d[:, 2].rearrange("b c h w -> c b (h w)"),
        )
        # t3..t7 halves across both HWDGE queues
        for t in range(3, T):
            for h in range(2):
                eng = nc.sync if h == 0 else nc.scalar
                eng.dma_start(
                    out=xts[t][:, h * FW // 2 : (h + 1) * FW // 2].rearrange(
                        "c (b f) -> c b f", b=2
                    ),
                    in_=x_d[2 * h : 2 * h + 2, t].rearrange(
                        "b c h w -> c b (h w)"
                    ),
                )

        # ---- recurrence ----------------------------------------------
        m_prev = None
        for t in range(T):
            th = float(2.0**t)
            xt = xts[t]
            m_cur = ms[t % 3]
            if t == 0:
                for b in range(B_LOC):
                    # fused copy+reset per quarter: M = (x_0 <= 1)*x_0
                    nc.vector.scalar_tensor_tensor(
                        bslice(m_cur, b), bslice(xt, b), 1.0, bslice(xt, b),
                        AluOpType.is_le, AluOpType.mult,
                    )
                pre = xt
            elif t == 1:
                for b in range(B_LOC):
                    # quarter-grain acc so compute tracks the DMA stream
                    nc.vector.scalar_tensor_tensor(
                        bslice(m_cur, b), bslice(xt, b), th,
                        bslice(m_prev, b), AluOpType.mult, AluOpType.add,
                    )
                pre = m_cur
            else:
                halves = 2 if t == T - 1 else 1
                HS = FW // halves
                for h in range(halves):
                    hs = slice(h * HS, (h + 1) * HS)
                    nc.vector.scalar_tensor_tensor(
                        m_cur[:, hs], xt[:, hs], th, m_prev[:, hs],
                        AluOpType.mult, AluOpType.add,
                    )
                pre = m_cur
            # spike u8, single ACT pass, off the recurrence path
            halves = 2 if t == T - 1 else 1
            HS = FW // halves
            for h in range(halves):
                s = sp.tile([C, HS], U8, name=f"s{halves}")
                nc.scalar.activation(
                    s, pre[:, h * HS : (h + 1) * HS], SIGN,
                    bias=-1.0, scale=1.0 / th,
                )
                nb = B_LOC // halves
                nc.gpsimd.dma_start(
                    out=o_d[h * nb : (h + 1) * nb, t].rearrange(
                        "b c h w -> c b (h w)"
                    ),
                    in_=s.rearrange("c (b f) -> c b f", b=nb),
                )
            if t < T - 1:
                # fused maskless reset into the next ping-pong tile
                m_rst = ms[(t + 1) % 3]
                if t == 0:
                    for b in range(B_LOC):
                        nc.vector.scalar_tensor_tensor(
                            bslice(m_rst, b), bslice(m_cur, b), th,
                            bslice(m_cur, b), AluOpType.is_le, AluOpType.mult,
                        )
                else:
                    nc.vector.scalar_tensor_tensor(
                        m_rst, m_cur, th, m_cur,
                        AluOpType.is_le, AluOpType.mult,
                    )
                m_prev = m_rst


def build_nc():
    """Build + compile the per-core Bass program (cached)."""
    if "nc" in _NC_CACHE:
        return _NC_CACHE["nc"]
    nc = bacc.Bacc(
        "TRN2",
        target_bir_lowering=False,
        debug=False,
        enable_asserts=False,
        num_devices=N_CORES,
    )
    x_d = nc.dram_tensor("x", [B_LOC, T, C, H, W], FP32, kind="ExternalInput").ap()
    o_d = nc.dram_tensor("out", [B_LOC, T, C, H, W], U8, kind="ExternalOutput").ap()
    # register the -1.0 bias constant (memset in the preamble)
    th_t = nc.alloc_sbuf_tensor("const-float32--1.0", [C, 1], FP32)
    nc.gpsimd.memset(th_t.ap(), -1.0)
    nc.const_aps.aps[(FP32, -1.0)] = th_t.ap()
    with tile.TileContext(nc) as tc:
        _emit(tc, x_d, o_d)
    nc.compile()
    _NC_CACHE["nc"] = nc
    return nc


def make_in_maps(x: np.ndarray) -> list[dict[str, np.ndarray]]:
    assert x.shape == (B, T, C, H, W) and x.dtype == np.float32, (x.shape, x.dtype)
    return [
        {"x": np.ascontiguousarray(x[i * B_LOC : (i + 1) * B_LOC])}
        for i in range(N_CORES)
    ]


def kernel(x: np.ndarray) -> np.ndarray:
    x = np.asarray(x, dtype=np.float32)
    nc = build_nc()
    res = run_bass_kernel_spmd(nc, make_in_maps(x), list(range(N_CORES)))
    return np.concatenate([r["out"] for r in res.results], axis=0).astype(np.float32)
